# revision 2
# baseline (speedup 1.0000x reference)
"""Trainium2 Bass kernel for nn_MemoryCore (retrieval KNN min-distance).

Problem: embedding [8192, 512], memory_bank [65536, 512] (fp32) ->
patch_scores [8192, 1] = min over the bank of euclidean distance.

Strategy (8 NeuronCores, SPMD):
  - Shard the memory bank (M axis) 8 ways; every core sees all queries.
  - Per core: psum[m, n] = (-2*bank_shard) @ emb.T via PE (float32r,
    1 cyc/row), fused running min over m-tiles on DVE via
    scalar_tensor_tensor: RM = min(psum + m_sq[m], RM).
  - Epilogue per 512-query block: PE-transpose RM, reduce_min over the
    free axis, sqrt(min + x_sq) on ACT -> per-core local min distances.
  - Host: elementwise min across the 8 cores.
"""
import numpy as np
import concourse.bacc as bacc
import concourse.mybir as mybir
import concourse.tile as tile
from concourse.bass_utils import run_bass_kernel_spmd
from concourse.masks import make_identity

N_CORES = 8
N, M, D = 8192, 65536, 512
MS = M // N_CORES       # 8192 bank rows per core
MSB = 1024              # bank chunk width (columns) per persistent tile
PSUM_BUFS = 6
BIG = 1e30
DT = mybir.dt.float32r  # TF32-like matmul: 4x faster than fp32, ~1e-4 rel err

_CACHE = {}


def _build_kernel():
    K = D // 128            # contraction chunks
    NB = N // 512           # query blocks (free axis)
    MT = MS // 128          # bank tiles (partitions)
    NMSB = MS // MSB
    mt_per_chunk = MSB // 128

    nc = bacc.Bacc("TRN2", target_bir_lowering=False, debug=False,
                   num_devices=N_CORES)

    embT_d = nc.dram_tensor("embT", [D, N], DT, kind="ExternalInput")
    bankT_d = nc.dram_tensor("bankT", [D, MS], DT, kind="ExternalInput")
    msq_d = nc.dram_tensor("msq", [128, MT], mybir.dt.float32, kind="ExternalInput")
    xsq_d = nc.dram_tensor("xsq", [128, N // 128], mybir.dt.float32, kind="ExternalInput")
    out_d = nc.dram_tensor("out", [128, N // 128], mybir.dt.float32, kind="ExternalOutput")

    with tile.TileContext(nc) as tc:
        with (
            tc.tile_pool(name="persist", bufs=1) as persist,
            tc.tile_pool(name="emb", bufs=2) as embp,
            tc.tile_pool(name="rmp", bufs=2) as rmp,
            tc.tile_pool(name="small", bufs=4) as small,
            tc.tile_pool(name="psum", bufs=PSUM_BUFS, space="PSUM") as psum,
            tc.tile_pool(name="psum_t", bufs=2, space="PSUM") as psum_t,
        ):
            msq = persist.tile([128, MT], mybir.dt.float32, tag="msq")
            nc.gpsimd.dma_start(msq[:], msq_d[:])
            xsq = persist.tile([128, N // 128], mybir.dt.float32, tag="xsq")
            nc.gpsimd.dma_start(xsq[:], xsq_d[:])
            out_s = persist.tile([128, N // 128], mybir.dt.float32, tag="outs")
            ident = persist.tile([128, 128], mybir.dt.float32, tag="ident")
            make_identity(nc, ident)

            def load_emb(nb):
                t = embp.tile([128, K, 512], DT, tag="embt")
                for k in range(K):
                    nc.gpsimd.dma_start(
                        t[:, k, :],
                        embT_d[k * 128:(k + 1) * 128, nb * 512:(nb + 1) * 512])
                return t

            emb_next = load_emb(0)

            bank_t = [[None] * NMSB for _ in range(K)]
            for j in range(NMSB):
                for k in range(K):
                    t = persist.tile([128, MSB], DT, tag=f"bank{k}_{j}")
                    nc.sync.dma_start(
                        t[:], bankT_d[k * 128:(k + 1) * 128, j * MSB:(j + 1) * MSB])
                    bank_t[k][j] = t

            for nb in range(NB):
                emb_t = emb_next
                if nb + 1 < NB:
                    emb_next = load_emb(nb + 1)
                rm = rmp.tile([128, 512], mybir.dt.float32, tag="rm")
                nc.vector.memset(rm[:], BIG)
                for mt in range(MT):
                    j, jj = mt // mt_per_chunk, mt % mt_per_chunk
                    ps = psum.tile([128, 512], mybir.dt.float32, tag="ps")
                    for k in range(K):
                        nc.tensor.matmul(
                            ps[:],
                            bank_t[k][j][:, jj * 128:(jj + 1) * 128],
                            emb_t[:, k, :],
                            start=(k == 0),
                            stop=(k == K - 1),
                        )
                    # RM = min(psum + m_sq[m], RM)  (one DVE op, reads PSUM)
                    nc.vector.scalar_tensor_tensor(
                        out=rm[:],
                        in0=ps[:],
                        scalar=msq[:, mt:mt + 1],
                        in1=rm[:],
                        op0=mybir.AluOpType.add,
                        op1=mybir.AluOpType.min,
                    )
                for q in range(4):  # cross-partition min per 128-query chunk
                    pt = psum_t.tile([128, 128], mybir.dt.float32, tag="pt")
                    nc.tensor.transpose(pt[:], rm[:, q * 128:(q + 1) * 128], ident[:])
                    mn = small.tile([128, 1], mybir.dt.float32, tag="mn")
                    nc.vector.tensor_reduce(
                        out=mn[:], in_=pt[:], axis=mybir.AxisListType.X,
                        op=mybir.AluOpType.min)
                    col = nb * 4 + q
                    nc.scalar.activation(
                        out=out_s[:, col:col + 1],
                        in_=mn[:],
                        func=mybir.ActivationFunctionType.Sqrt,
                        bias=xsq[:, col:col + 1],
                        scale=1.0,
                    )
            nc.sync.dma_start(out_d[:], out_s[:])

    nc.compile()
    return nc


def kernel(embedding: np.ndarray, memory_bank: np.ndarray) -> np.ndarray:
    emb = np.asarray(embedding, dtype=np.float32)
    bank = np.asarray(memory_bank, dtype=np.float32)
    assert emb.shape == (N, D) and bank.shape == (M, D)

    if "nc" not in _CACHE:
        _CACHE["nc"] = _build_kernel()
    nc = _CACHE["nc"]

    embT = np.ascontiguousarray(emb.T)
    x_sq = np.einsum("nd,nd->n", emb, emb, dtype=np.float64).astype(np.float32)
    xsq = np.ascontiguousarray(x_sq.reshape(N // 128, 128).T)

    in_maps = []
    for c in range(N_CORES):
        shard = bank[c * MS:(c + 1) * MS]
        bankT = np.ascontiguousarray((-2.0 * shard).T)
        m_sq = np.einsum("md,md->m", shard, shard, dtype=np.float64).astype(np.float32)
        msq = np.ascontiguousarray(m_sq.reshape(MS // 128, 128).T)
        in_maps.append({"embT": embT, "bankT": bankT, "msq": msq, "xsq": xsq})

    _CACHE["last_in_maps"] = in_maps
    res = run_bass_kernel_spmd(nc, in_maps, core_ids=list(range(N_CORES)))

    # gather: each core returns [128, N/128] local min distances; min over cores
    per_core = np.stack([res.results[c]["out"].T.reshape(N) for c in range(N_CORES)])
    return per_core.min(axis=0).reshape(N, 1).astype(np.float32)
